# revision 15
# baseline (speedup 1.0000x reference)
"""Trainium2 Bass kernel for nn_CrossAttention (N=16,Q=4096,C=77,D=512,Dc=768,H=8,S=64).

Sharding: data-parallel over batch N across 8 cores (2 batches/core, no collectives).
Per-core kernel computes full multi-head cross-attention for its 2 batches.

Layout strategy (everything contracted on partitions, transposed "T" layouts):
  queryT[d, i]   <- PE transpose of query tiles
  qT[hs, i]      <- Wq_pair.T @ queryT            (q projection, pair = 2 heads)
  scoresT[c, i]  <- kT_h.T @ qT_h                 (K pre-scaled by 1/sqrt(S))
  expT[c, i]     <- exp(scoresT)   (no max-sub: scores are O(1) by construction)
  avT[s, i]      <- v_h.T @ expT_h  (col-tiled pair -> [128, i] psum)
  colB[s, i]     <- ones.T @ expT_h (broadcast column-sum, col-tiled pair)
  attnT          <- avT * recip(colB)             (softmax normalize)
  out[i, d]      <- attnT.T @ Wo + ones.T @ bo    (o projection + bias)

Matmul dtype knob: float32r (1 cyc/row at N>=256) vs float32 (4 cyc/row).
float32r requires every SBUF operand to be written by an op that rounds to
f32r — DMA-loaded weights get a one-time rounding copy; activations/evac
copies just write with f32r output dtype.
"""

import sys

if "/opt/trn_rl_repo" not in sys.path:
    sys.path.insert(0, "/opt/trn_rl_repo")

import numpy as np

import concourse.bass as bass
import concourse.tile as tile
from concourse import bacc, mybir
from concourse.bass_utils import run_bass_kernel_spmd
from concourse.masks import make_identity

# Problem shapes (hardcoded per spec)
N, Q, C = 16, 4096, 77
D, DC, H, S = 512, 768, 8, 64
HS = H * S  # 512
N_CORES = 8
NB = N // N_CORES  # batches per core = 2
P = 128
CHUNK = 512  # i-chunk (PSUM free dim)
N_CHUNKS = Q // CHUNK  # 8
IT_PER_CHUNK = CHUNK // P  # 4
N_PAIRS = H // 2  # 4 head-pairs
KT_D = D // P  # 4 k-tiles over D
KT_DC = DC // P  # 6 k-tiles over Dc

F32 = mybir.dt.float32
F32R = mybir.dt.float32r


def build_kernel(use_f32r=True, with_bias=True, pools=None):
    """Build the single-core Bass program (same program on all 8 cores)."""
    nc = bacc.Bacc("TRN2", target_bir_lowering=False, debug=False,
                   num_devices=N_CORES)

    query = nc.dram_tensor("query", [NB, Q, D], F32, kind="ExternalInput").ap()
    context = nc.dram_tensor("context", [NB, C, DC], F32, kind="ExternalInput").ap()
    Wq = nc.dram_tensor("Wq", [D, HS], F32, kind="ExternalInput").ap()
    Wk = nc.dram_tensor("Wk", [DC, HS], F32, kind="ExternalInput").ap()
    Wv = nc.dram_tensor("Wv", [DC, HS], F32, kind="ExternalInput").ap()
    Wo = nc.dram_tensor("Wo", [HS, D], F32, kind="ExternalInput").ap()
    bo = nc.dram_tensor("bo", [D], F32, kind="ExternalInput").ap()
    out = nc.dram_tensor("out", [NB, Q, D], F32, kind="ExternalOutput").ap()

    with tile.TileContext(nc) as tc:
        _emit(nc, tc, query, context, Wq, Wk, Wv, Wo, bo, out, use_f32r, with_bias, pools or {})
    nc.compile()
    return nc


def _emit(nc, tc, query, context, Wq, Wk, Wv, Wo, bo, out, use_f32r, with_bias, pools):
    from contextlib import ExitStack

    MMDT = F32R if use_f32r else F32  # dtype of big-matmul SBUF operands
    # attention-block dtype: f32r forbids dst start_partition != 0 (col-tiled
    # h1) and odd innermost counts, so the small attention matmuls use bf16
    # in the fast config.
    ATTDT = mybir.dt.bfloat16 if use_f32r else F32
    CPAD = C + 1 if use_f32r else C  # even innermost for kT matmul under f32r

    ctx = ExitStack()
    with ctx:
        consts = ctx.enter_context(tc.tile_pool(name="consts", bufs=1))
        wpool = ctx.enter_context(tc.tile_pool(name="weights", bufs=1))
        stage = ctx.enter_context(tc.tile_pool(name="stage", bufs=1))
        ctxp = ctx.enter_context(tc.tile_pool(name="ctxphase", bufs=2))
        qin = ctx.enter_context(tc.tile_pool(name="qin", bufs=pools.get("qin", 2)))
        qtp = ctx.enter_context(tc.tile_pool(name="qtp", bufs=pools.get("qtp", 2)))
        qtc = ctx.enter_context(tc.tile_pool(name="qtc", bufs=pools.get("qtc", 2)))
        expp = ctx.enter_context(tc.tile_pool(name="expp", bufs=pools.get("expp", 2)))
        attp = ctx.enter_context(tc.tile_pool(name="attp", bufs=pools.get("attp", 2)))
        recp = ctx.enter_context(tc.tile_pool(name="recp", bufs=pools.get("recp", 2)))
        outp = ctx.enter_context(tc.tile_pool(name="outp", bufs=pools.get("outp", 2)))

        pg = lambda k, d: pools.get(k, d)
        ps_tp = ctx.enter_context(tc.tile_pool(name="ps_tq", bufs=pg("tq", 2), space="PSUM"))
        ps_qp = ps_tp
        if pg("merge_att", 0):
            ps_sc = ctx.enter_context(tc.tile_pool(name="ps_att", bufs=pg("att", 4), space="PSUM"))
            ps_av = ps_sc
            ps_cs = ps_sc
        else:
            ps_sc = ctx.enter_context(tc.tile_pool(name="ps_sc", bufs=pg("sc", 2), space="PSUM"))
            ps_av = ctx.enter_context(tc.tile_pool(name="ps_av", bufs=pg("av", 1), space="PSUM"))
            ps_cs = ctx.enter_context(tc.tile_pool(name="ps_cs", bufs=pg("cs", 1), space="PSUM"))
        ps_o = ctx.enter_context(tc.tile_pool(name="ps_o", bufs=pg("o", 2), space="PSUM"))

        BF16_TP = bool(pools.get("bf16_tp", 1)) and use_f32r
        # ---- constants (once) ----
        ident = consts.tile([P, P], F32)  # fp32: feeds fp32 transposes only
        make_identity(nc, ident[:])
        if BF16_TP:
            ident_bf = consts.tile([P, P], mybir.dt.bfloat16)
            nc.vector.tensor_copy(ident_bf[:], ident[:])
        ones77 = consts.tile([C, S], ATTDT)
        ones77_f32 = consts.tile([C, S], F32)
        nc.gpsimd.memset(ones77_f32[:], 1.0)
        nc.vector.tensor_copy(ones77[:], ones77_f32[:])
        onesrow = consts.tile([1, P], MMDT)
        onesrow_f32 = consts.tile([1, P], F32)
        nc.gpsimd.memset(onesrow_f32[:], 1.0)
        nc.vector.tensor_copy(onesrow[:], onesrow_f32[:])
        zpad = consts.tile([P, KT_DC], F32)
        nc.gpsimd.memset(zpad[:], 0.0)
        bo_sb = consts.tile([1, D], MMDT)
        bo_stage = consts.tile([1, D], F32)
        nc.sync.dma_start(bo_stage[:], bo[None, :])
        nc.vector.tensor_copy(bo_sb[:], bo_stage[:])

        # ---- weights: DMA to fp32 staging, rounding-copy to MMDT ----
        wq_sb = wpool.tile([P, KT_D, HS], mybir.dt.bfloat16 if BF16_TP else MMDT)
        wk_sb = wpool.tile([P, KT_DC, HS], MMDT)
        wv_sb = wpool.tile([P, KT_DC, HS], MMDT)
        wo_sb = wpool.tile([P, KT_D, D], MMDT)
        scale = float(S) ** -0.5
        for w_sb, w_dram, kt_n, do_scale in (
            (wq_sb, Wq, KT_D, False),
            (wk_sb, Wk, KT_DC, True),
            (wv_sb, Wv, KT_DC, False),
            (wo_sb, Wo, KT_D, False),
        ):
            st = stage.tile([P, kt_n, HS], F32, tag="wstage")
            nc.sync.dma_start(st[:], w_dram.rearrange("(kt p) n -> p kt n", p=P))
            if do_scale:
                nc.vector.tensor_scalar_mul(w_sb[:], st[:], scale)
            else:
                nc.vector.tensor_copy(w_sb[:], st[:])

        for b in range(NB):
            # ---- context phase: ctxT, kT, v ----
            ctx_nat = ctxp.tile([C, DC], F32, tag="ctx_nat")
            nc.sync.dma_start(ctx_nat[:], context[b])
            ctxT = ctxp.tile([P, KT_DC, CPAD], MMDT, tag="ctxT")
            if CPAD != C:
                nc.vector.tensor_copy(ctxT[:, :, C:], zpad[:, :, None])
            for kt in range(KT_DC):
                pt = ps_tp.tile([P, CHUNK], F32, tag="tp")
                nc.tensor.transpose(
                    pt[:, :C],
                    ctx_nat[:, kt * P:(kt + 1) * P],
                    ident[:C, :C],
                )
                nc.vector.tensor_copy(ctxT[:, kt, :C], pt[:, :C])

            kT = ctxp.tile([P, N_PAIRS, C], MMDT, tag="kT")
            v_sb = ctxp.tile([C, HS], ATTDT, tag="v_sb")
            for hp in range(N_PAIRS):
                pk = ps_sc.tile([P, CHUNK], F32, tag="att" if pg("merge_att", 0) else "sc")
                for kt in range(KT_DC):
                    nc.tensor.matmul(
                        pk[:, :CPAD],
                        wk_sb[:, kt, hp * P:(hp + 1) * P],
                        ctxT[:, kt, :],
                        start=(kt == 0), stop=(kt == KT_DC - 1),
                    )
                nc.vector.tensor_copy(kT[:, hp, :], pk[:, :C])
                pv = ps_av.tile([P, CHUNK], F32, tag="att" if pg("merge_att", 0) else "av")
                for kt in range(KT_DC):
                    nc.tensor.matmul(
                        pv[:C, :P],
                        ctxT[:, kt, :C],
                        wv_sb[:, kt, hp * P:(hp + 1) * P],
                        start=(kt == 0), stop=(kt == KT_DC - 1),
                    )
                nc.vector.tensor_copy(v_sb[:, hp * P:(hp + 1) * P], pv[:C, :P])

            # ---- main loop over i-chunks ----
            for ch in range(N_CHUNKS):
                i0 = ch * CHUNK
                q_raw = qin.tile([P, IT_PER_CHUNK, CHUNK],
                                 mybir.dt.bfloat16 if BF16_TP else F32,
                                 tag="q_raw")
                if BF16_TP:
                    nc.gpsimd.dma_start(
                        q_raw[:],
                        query[b, i0:i0 + CHUNK, :].rearrange("(t p) c -> p t c", p=P),
                    )
                else:
                    nc.sync.dma_start(
                        q_raw[:],
                        query[b, i0:i0 + CHUNK, :].rearrange("(t p) c -> p t c", p=P),
                    )
                # transpose chunk -> queryT_c [128(d), KT_D, CHUNK(i)]
                queryT_c = qtp.tile([P, KT_D, CHUNK],
                                    mybir.dt.bfloat16 if BF16_TP else MMDT, tag="queryT")
                for it in range(IT_PER_CHUNK):
                    pt = ps_tp.tile([P, CHUNK], mybir.dt.bfloat16 if BF16_TP else F32,
                                    tag="tp")
                    for kt in range(KT_D):
                        nc.tensor.transpose(
                            pt[:, kt * P:(kt + 1) * P],
                            q_raw[:, it, kt * P:(kt + 1) * P],
                            ident_bf[:] if BF16_TP else ident[:],
                        )
                    nc.vector.tensor_copy(
                        queryT_c[:, :, it * P:(it + 1) * P],
                        pt[:].rearrange("p (kt i) -> p kt i", kt=KT_D),
                    )
                # q projection -> qT_c [128(2 heads' s), N_PAIRS, CHUNK]
                qT_c = qtc.tile([P, N_PAIRS, CHUNK], MMDT, tag="qT")
                for hp in range(N_PAIRS):
                    pq = ps_qp.tile([P, CHUNK], F32, tag="tp")
                    for kt in range(KT_D):
                        nc.tensor.matmul(
                            pq[:],
                            wq_sb[:, kt, hp * P:(hp + 1) * P],
                            queryT_c[:, kt, :],
                            start=(kt == 0), stop=(kt == KT_D - 1),
                        )
                    nc.scalar.copy(qT_c[:, hp, :], pq[:])

                # attention per head-pair
                expT_c = expp.tile([C, H, CHUNK], ATTDT, tag="expT")
                attnT_c = [attp.tile([P, CHUNK], MMDT, tag=f"attnT{hp}",
                                     name=f"attnT{hp}")
                           for hp in range(N_PAIRS)]
                for hp in range(N_PAIRS):
                    h0, h1 = 2 * hp, 2 * hp + 1
                    # scoresT: row-tiled pair (K=64 each at partitions 0/64)
                    ps0 = ps_sc.tile([P, CHUNK], F32, tag="att" if pg("merge_att", 0) else "sc")
                    ps1 = ps_sc.tile([P, CHUNK], F32, tag="att" if pg("merge_att", 0) else "sc")
                    nc.tensor.matmul(
                        ps0[:C, :], kT[0:S, hp, :], qT_c[0:S, hp, :],
                        start=True, stop=True,
                    )
                    nc.tensor.matmul(
                        ps1[:C, :], kT[S:P, hp, :], qT_c[S:P, hp, :],
                        start=True, stop=True,
                    )
                    nc.scalar.activation(
                        expT_c[:, h0, :], ps0[:C, :],
                        mybir.ActivationFunctionType.Exp,
                    )
                    nc.scalar.activation(
                        expT_c[:, h1, :], ps1[:C, :],
                        mybir.ActivationFunctionType.Exp,
                    )
                    # av + colsum-broadcast, col-tiled pairs
                    pav = ps_av.tile([P, CHUNK], F32, tag="att" if pg("merge_att", 0) else "av")
                    pcs = ps_cs.tile([P, CHUNK], F32, tag="att" if pg("merge_att", 0) else "cs")
                    nc.tensor.matmul(
                        pav[0:S, :], v_sb[:, h0 * S:(h0 + 1) * S],
                        expT_c[:, h0, :],
                        start=True, stop=True, tile_position=(0, 0),
                    )
                    nc.tensor.matmul(
                        pav[S:P, :], v_sb[:, h1 * S:(h1 + 1) * S],
                        expT_c[:, h1, :],
                        start=True, stop=True, tile_position=(0, S),
                    )
                    nc.tensor.matmul(
                        pcs[0:S, :], ones77[:], expT_c[:, h0, :],
                        start=True, stop=True, tile_position=(0, 0),
                    )
                    nc.tensor.matmul(
                        pcs[S:P, :], ones77[:], expT_c[:, h1, :],
                        start=True, stop=True, tile_position=(0, S),
                    )
                    recipB = recp.tile([P, CHUNK], F32, tag="recipB")
                    nc.vector.reciprocal_approx_fast(recipB[:], pcs[:])
                    nc.vector.tensor_tensor(
                        attnT_c[hp][:], pav[:], recipB[:],
                        mybir.AluOpType.mult,
                    )

                # o-projection (+ bias via K=1 ones-row matmul)
                outc = outp.tile([P, IT_PER_CHUNK, D], F32, tag="outc")
                for it in range(IT_PER_CHUNK):
                    po = ps_o.tile([P, D], F32, tag="o")
                    for kt in range(KT_D):
                        nc.tensor.matmul(
                            po[:],
                            attnT_c[kt][:, it * P:(it + 1) * P],
                            wo_sb[:, kt, :],
                            start=(kt == 0),
                            stop=(not with_bias and kt == KT_D - 1),
                        )
                    if with_bias:
                        nc.tensor.matmul(
                            po[:], onesrow[:], bo_sb[:],
                            start=False, stop=True,
                        )
                    nc.scalar.copy(outc[:, it, :], po[:])
                nc.sync.dma_start(
                    out[b, i0:i0 + CHUNK, :].rearrange("(t p) c -> p t c", p=P),
                    outc[:],
                )


_CACHE = {}


def _get_nc(use_f32r=True, with_bias=True):
    key = (use_f32r, with_bias)
    if key not in _CACHE:
        _CACHE[key] = build_kernel(use_f32r, with_bias)
    return _CACHE[key]


def kernel(query, context, Wq, Wk, Wv, Wo, bo, _use_f32r=True):
    query = np.ascontiguousarray(np.asarray(query, dtype=np.float32))
    context = np.ascontiguousarray(np.asarray(context, dtype=np.float32))
    Wq = np.asarray(Wq, dtype=np.float32).reshape(D, HS)
    Wk = np.asarray(Wk, dtype=np.float32).reshape(DC, HS)
    Wv = np.asarray(Wv, dtype=np.float32).reshape(DC, HS)
    Wo = np.asarray(Wo, dtype=np.float32).reshape(HS, D)
    bo = np.asarray(bo, dtype=np.float32).reshape(D)

    nc = _get_nc(use_f32r=_use_f32r, with_bias=bool(np.any(bo)))
    in_maps = []
    for c in range(N_CORES):
        sl = slice(c * NB, (c + 1) * NB)
        in_maps.append({
            "query": np.ascontiguousarray(query[sl]),
            "context": np.ascontiguousarray(context[sl]),
            "Wq": Wq, "Wk": Wk, "Wv": Wv, "Wo": Wo, "bo": bo,
        })
    res = run_bass_kernel_spmd(nc, in_maps, core_ids=list(range(N_CORES)))
    return np.concatenate([res.results[c]["out"] for c in range(N_CORES)], axis=0)


# revision 16
# speedup vs baseline: 1.0062x; 1.0062x over previous
"""Trainium2 Bass kernel for nn_CrossAttention (N=16,Q=4096,C=77,D=512,Dc=768,H=8,S=64).

Sharding: data-parallel over batch N across 8 cores (2 batches/core, no collectives).
Per-core kernel computes full multi-head cross-attention for its 2 batches.

Layout strategy (everything contracted on partitions, transposed "T" layouts):
  queryT[d, i]   <- PE transpose of query tiles
  qT[hs, i]      <- Wq_pair.T @ queryT            (q projection, pair = 2 heads)
  scoresT[c, i]  <- kT_h.T @ qT_h                 (K pre-scaled by 1/sqrt(S))
  expT[c, i]     <- exp(scoresT)   (no max-sub: scores are O(1) by construction)
  avT[s, i]      <- v_h.T @ expT_h  (col-tiled pair -> [128, i] psum)
  colB[s, i]     <- ones.T @ expT_h (broadcast column-sum, col-tiled pair)
  attnT          <- avT * recip(colB)             (softmax normalize)
  out[i, d]      <- attnT.T @ Wo + ones.T @ bo    (o projection + bias)

Matmul dtype knob: float32r (1 cyc/row at N>=256) vs float32 (4 cyc/row).
float32r requires every SBUF operand to be written by an op that rounds to
f32r — DMA-loaded weights get a one-time rounding copy; activations/evac
copies just write with f32r output dtype.
"""

import sys

if "/opt/trn_rl_repo" not in sys.path:
    sys.path.insert(0, "/opt/trn_rl_repo")

import numpy as np

import concourse.bass as bass
import concourse.tile as tile
from concourse import bacc, mybir
from concourse.bass_utils import run_bass_kernel_spmd
from concourse.masks import make_identity

# Problem shapes (hardcoded per spec)
N, Q, C = 16, 4096, 77
D, DC, H, S = 512, 768, 8, 64
HS = H * S  # 512
N_CORES = 8
NB = N // N_CORES  # batches per core = 2
P = 128
CHUNK = 512  # i-chunk (PSUM free dim)
N_CHUNKS = Q // CHUNK  # 8
IT_PER_CHUNK = CHUNK // P  # 4
N_PAIRS = H // 2  # 4 head-pairs
KT_D = D // P  # 4 k-tiles over D
KT_DC = DC // P  # 6 k-tiles over Dc

F32 = mybir.dt.float32
F32R = mybir.dt.float32r


def build_kernel(use_f32r=True, with_bias=True, pools=None):
    """Build the single-core Bass program (same program on all 8 cores)."""
    nc = bacc.Bacc("TRN2", target_bir_lowering=False, debug=False,
                   num_devices=N_CORES)

    query = nc.dram_tensor("query", [NB, Q, D], F32, kind="ExternalInput").ap()
    context = nc.dram_tensor("context", [NB, C, DC], F32, kind="ExternalInput").ap()
    Wq = nc.dram_tensor("Wq", [D, HS], F32, kind="ExternalInput").ap()
    Wk = nc.dram_tensor("Wk", [DC, HS], F32, kind="ExternalInput").ap()
    Wv = nc.dram_tensor("Wv", [DC, HS], F32, kind="ExternalInput").ap()
    Wo = nc.dram_tensor("Wo", [HS, D], F32, kind="ExternalInput").ap()
    bo = nc.dram_tensor("bo", [D], F32, kind="ExternalInput").ap()
    out = nc.dram_tensor("out", [NB, Q, D], F32, kind="ExternalOutput").ap()

    with tile.TileContext(nc) as tc:
        _emit(nc, tc, query, context, Wq, Wk, Wv, Wo, bo, out, use_f32r, with_bias, pools or {})
    nc.compile()
    return nc


def _emit(nc, tc, query, context, Wq, Wk, Wv, Wo, bo, out, use_f32r, with_bias, pools):
    from contextlib import ExitStack

    MMDT = F32R if use_f32r else F32  # dtype of big-matmul SBUF operands
    # attention-block dtype: f32r forbids dst start_partition != 0 (col-tiled
    # h1) and odd innermost counts, so the small attention matmuls use bf16
    # in the fast config.
    ATTDT = mybir.dt.bfloat16 if use_f32r else F32
    CPAD = C + 1 if use_f32r else C  # even innermost for kT matmul under f32r

    ctx = ExitStack()
    with ctx:
        consts = ctx.enter_context(tc.tile_pool(name="consts", bufs=1))
        wpool = ctx.enter_context(tc.tile_pool(name="weights", bufs=1))
        stage = ctx.enter_context(tc.tile_pool(name="stage", bufs=1))
        ctxp = ctx.enter_context(tc.tile_pool(name="ctxphase", bufs=2))
        qin = ctx.enter_context(tc.tile_pool(name="qin", bufs=pools.get("qin", 2)))
        qtp = ctx.enter_context(tc.tile_pool(name="qtp", bufs=pools.get("qtp", 2)))
        qtc = ctx.enter_context(tc.tile_pool(name="qtc", bufs=pools.get("qtc", 2)))
        expp = ctx.enter_context(tc.tile_pool(name="expp", bufs=pools.get("expp", 2)))
        attp = ctx.enter_context(tc.tile_pool(name="attp", bufs=pools.get("attp", 2)))
        recp = ctx.enter_context(tc.tile_pool(name="recp", bufs=pools.get("recp", 2)))
        outp = ctx.enter_context(tc.tile_pool(name="outp", bufs=pools.get("outp", 2)))

        pg = lambda k, d: pools.get(k, d)
        ps_tp = ctx.enter_context(tc.tile_pool(name="ps_tq", bufs=pg("tq", 2), space="PSUM"))
        ps_qp = ps_tp
        if pg("merge_att", 0):
            ps_sc = ctx.enter_context(tc.tile_pool(name="ps_att", bufs=pg("att", 4), space="PSUM"))
            ps_av = ps_sc
            ps_cs = ps_sc
        else:
            ps_sc = ctx.enter_context(tc.tile_pool(name="ps_sc", bufs=pg("sc", 2), space="PSUM"))
            ps_av = ctx.enter_context(tc.tile_pool(name="ps_av", bufs=pg("av", 2), space="PSUM"))
            ps_cs = ctx.enter_context(tc.tile_pool(name="ps_cs", bufs=pg("cs", 1), space="PSUM"))
        ps_o = ctx.enter_context(tc.tile_pool(name="ps_o", bufs=pg("o", 1), space="PSUM"))

        BF16_TP = bool(pools.get("bf16_tp", 1)) and use_f32r
        # ---- constants (once) ----
        ident = consts.tile([P, P], F32)  # fp32: feeds fp32 transposes only
        make_identity(nc, ident[:])
        if BF16_TP:
            ident_bf = consts.tile([P, P], mybir.dt.bfloat16)
            nc.vector.tensor_copy(ident_bf[:], ident[:])
        ones77 = consts.tile([C, S], ATTDT)
        ones77_f32 = consts.tile([C, S], F32)
        nc.gpsimd.memset(ones77_f32[:], 1.0)
        nc.vector.tensor_copy(ones77[:], ones77_f32[:])
        onesrow = consts.tile([1, P], MMDT)
        onesrow_f32 = consts.tile([1, P], F32)
        nc.gpsimd.memset(onesrow_f32[:], 1.0)
        nc.vector.tensor_copy(onesrow[:], onesrow_f32[:])
        zpad = consts.tile([P, KT_DC], F32)
        nc.gpsimd.memset(zpad[:], 0.0)
        bo_sb = consts.tile([1, D], MMDT)
        bo_stage = consts.tile([1, D], F32)
        nc.sync.dma_start(bo_stage[:], bo[None, :])
        nc.vector.tensor_copy(bo_sb[:], bo_stage[:])

        # ---- weights: DMA to fp32 staging, rounding-copy to MMDT ----
        wq_sb = wpool.tile([P, KT_D, HS], mybir.dt.bfloat16 if BF16_TP else MMDT)
        wk_sb = wpool.tile([P, KT_DC, HS], MMDT)
        wv_sb = wpool.tile([P, KT_DC, HS], MMDT)
        wo_sb = wpool.tile([P, KT_D, D], MMDT)
        scale = float(S) ** -0.5
        for w_sb, w_dram, kt_n, do_scale in (
            (wq_sb, Wq, KT_D, False),
            (wk_sb, Wk, KT_DC, True),
            (wv_sb, Wv, KT_DC, False),
            (wo_sb, Wo, KT_D, False),
        ):
            st = stage.tile([P, kt_n, HS], F32, tag="wstage")
            nc.sync.dma_start(st[:], w_dram.rearrange("(kt p) n -> p kt n", p=P))
            if do_scale:
                nc.vector.tensor_scalar_mul(w_sb[:], st[:], scale)
            else:
                nc.vector.tensor_copy(w_sb[:], st[:])

        for b in range(NB):
            # ---- context phase: ctxT, kT, v ----
            ctx_nat = ctxp.tile([C, DC], F32, tag="ctx_nat")
            nc.sync.dma_start(ctx_nat[:], context[b])
            ctxT = ctxp.tile([P, KT_DC, CPAD], MMDT, tag="ctxT")
            if CPAD != C:
                nc.vector.tensor_copy(ctxT[:, :, C:], zpad[:, :, None])
            for kt in range(KT_DC):
                pt = ps_tp.tile([P, CHUNK], F32, tag="tp")
                nc.tensor.transpose(
                    pt[:, :C],
                    ctx_nat[:, kt * P:(kt + 1) * P],
                    ident[:C, :C],
                )
                nc.vector.tensor_copy(ctxT[:, kt, :C], pt[:, :C])

            kT = ctxp.tile([P, N_PAIRS, C], MMDT, tag="kT")
            v_sb = ctxp.tile([C, HS], ATTDT, tag="v_sb")
            for hp in range(N_PAIRS):
                pk = ps_sc.tile([P, CHUNK], F32, tag="att" if pg("merge_att", 0) else "sc")
                for kt in range(KT_DC):
                    nc.tensor.matmul(
                        pk[:, :CPAD],
                        wk_sb[:, kt, hp * P:(hp + 1) * P],
                        ctxT[:, kt, :],
                        start=(kt == 0), stop=(kt == KT_DC - 1),
                    )
                nc.vector.tensor_copy(kT[:, hp, :], pk[:, :C])
                pv = ps_av.tile([P, CHUNK], F32, tag="att" if pg("merge_att", 0) else "av")
                for kt in range(KT_DC):
                    nc.tensor.matmul(
                        pv[:C, :P],
                        ctxT[:, kt, :C],
                        wv_sb[:, kt, hp * P:(hp + 1) * P],
                        start=(kt == 0), stop=(kt == KT_DC - 1),
                    )
                nc.vector.tensor_copy(v_sb[:, hp * P:(hp + 1) * P], pv[:C, :P])

            # ---- main loop over i-chunks ----
            for ch in range(N_CHUNKS):
                i0 = ch * CHUNK
                q_raw = qin.tile([P, IT_PER_CHUNK, CHUNK],
                                 mybir.dt.bfloat16 if BF16_TP else F32,
                                 tag="q_raw")
                if BF16_TP:
                    nc.gpsimd.dma_start(
                        q_raw[:],
                        query[b, i0:i0 + CHUNK, :].rearrange("(t p) c -> p t c", p=P),
                    )
                else:
                    nc.sync.dma_start(
                        q_raw[:],
                        query[b, i0:i0 + CHUNK, :].rearrange("(t p) c -> p t c", p=P),
                    )
                # transpose chunk -> queryT_c [128(d), KT_D, CHUNK(i)]
                queryT_c = qtp.tile([P, KT_D, CHUNK],
                                    mybir.dt.bfloat16 if BF16_TP else MMDT, tag="queryT")
                for it in range(IT_PER_CHUNK):
                    pt = ps_tp.tile([P, CHUNK], mybir.dt.bfloat16 if BF16_TP else F32,
                                    tag="tp")
                    for kt in range(KT_D):
                        nc.tensor.transpose(
                            pt[:, kt * P:(kt + 1) * P],
                            q_raw[:, it, kt * P:(kt + 1) * P],
                            ident_bf[:] if BF16_TP else ident[:],
                        )
                    nc.vector.tensor_copy(
                        queryT_c[:, :, it * P:(it + 1) * P],
                        pt[:].rearrange("p (kt i) -> p kt i", kt=KT_D),
                    )
                # q projection -> qT_c [128(2 heads' s), N_PAIRS, CHUNK]
                qT_c = qtc.tile([P, N_PAIRS, CHUNK], MMDT, tag="qT")
                for hp in range(N_PAIRS):
                    pq = ps_qp.tile([P, CHUNK], F32, tag="tp")
                    for kt in range(KT_D):
                        nc.tensor.matmul(
                            pq[:],
                            wq_sb[:, kt, hp * P:(hp + 1) * P],
                            queryT_c[:, kt, :],
                            start=(kt == 0), stop=(kt == KT_D - 1),
                        )
                    nc.scalar.copy(qT_c[:, hp, :], pq[:])

                # attention per head-pair
                expT_c = expp.tile([C, H, CHUNK], ATTDT, tag="expT")
                attnT_c = [attp.tile([P, CHUNK], MMDT, tag=f"attnT{hp}",
                                     name=f"attnT{hp}")
                           for hp in range(N_PAIRS)]
                for hp in range(N_PAIRS):
                    h0, h1 = 2 * hp, 2 * hp + 1
                    # scoresT: row-tiled pair (K=64 each at partitions 0/64)
                    ps0 = ps_sc.tile([P, CHUNK], F32, tag="att" if pg("merge_att", 0) else "sc")
                    ps1 = ps_sc.tile([P, CHUNK], F32, tag="att" if pg("merge_att", 0) else "sc")
                    nc.tensor.matmul(
                        ps0[:C, :], kT[0:S, hp, :], qT_c[0:S, hp, :],
                        start=True, stop=True,
                    )
                    nc.tensor.matmul(
                        ps1[:C, :], kT[S:P, hp, :], qT_c[S:P, hp, :],
                        start=True, stop=True,
                    )
                    nc.scalar.activation(
                        expT_c[:, h0, :], ps0[:C, :],
                        mybir.ActivationFunctionType.Exp,
                    )
                    nc.scalar.activation(
                        expT_c[:, h1, :], ps1[:C, :],
                        mybir.ActivationFunctionType.Exp,
                    )
                    # av + colsum-broadcast, col-tiled pairs
                    pav = ps_av.tile([P, CHUNK], F32, tag="att" if pg("merge_att", 0) else "av")
                    pcs = ps_cs.tile([P, CHUNK], F32, tag="att" if pg("merge_att", 0) else "cs")
                    nc.tensor.matmul(
                        pav[0:S, :], v_sb[:, h0 * S:(h0 + 1) * S],
                        expT_c[:, h0, :],
                        start=True, stop=True, tile_position=(0, 0),
                    )
                    nc.tensor.matmul(
                        pav[S:P, :], v_sb[:, h1 * S:(h1 + 1) * S],
                        expT_c[:, h1, :],
                        start=True, stop=True, tile_position=(0, S),
                    )
                    nc.tensor.matmul(
                        pcs[0:S, :], ones77[:], expT_c[:, h0, :],
                        start=True, stop=True, tile_position=(0, 0),
                    )
                    nc.tensor.matmul(
                        pcs[S:P, :], ones77[:], expT_c[:, h1, :],
                        start=True, stop=True, tile_position=(0, S),
                    )
                    recipB = recp.tile([P, CHUNK], F32, tag="recipB")
                    nc.vector.reciprocal_approx_fast(recipB[:], pcs[:])
                    nc.vector.tensor_tensor(
                        attnT_c[hp][:], pav[:], recipB[:],
                        mybir.AluOpType.mult,
                    )

                # o-projection (+ bias via K=1 ones-row matmul)
                outc = outp.tile([P, IT_PER_CHUNK, D], F32, tag="outc")
                for it in range(IT_PER_CHUNK):
                    po = ps_o.tile([P, D], F32, tag="o")
                    for kt in range(KT_D):
                        nc.tensor.matmul(
                            po[:],
                            attnT_c[kt][:, it * P:(it + 1) * P],
                            wo_sb[:, kt, :],
                            start=(kt == 0),
                            stop=(not with_bias and kt == KT_D - 1),
                        )
                    if with_bias:
                        nc.tensor.matmul(
                            po[:], onesrow[:], bo_sb[:],
                            start=False, stop=True,
                        )
                    nc.scalar.copy(outc[:, it, :], po[:])
                nc.sync.dma_start(
                    out[b, i0:i0 + CHUNK, :].rearrange("(t p) c -> p t c", p=P),
                    outc[:],
                )


_CACHE = {}


def _get_nc(use_f32r=True, with_bias=True):
    key = (use_f32r, with_bias)
    if key not in _CACHE:
        _CACHE[key] = build_kernel(use_f32r, with_bias)
    return _CACHE[key]


def kernel(query, context, Wq, Wk, Wv, Wo, bo, _use_f32r=True):
    query = np.ascontiguousarray(np.asarray(query, dtype=np.float32))
    context = np.ascontiguousarray(np.asarray(context, dtype=np.float32))
    Wq = np.asarray(Wq, dtype=np.float32).reshape(D, HS)
    Wk = np.asarray(Wk, dtype=np.float32).reshape(DC, HS)
    Wv = np.asarray(Wv, dtype=np.float32).reshape(DC, HS)
    Wo = np.asarray(Wo, dtype=np.float32).reshape(HS, D)
    bo = np.asarray(bo, dtype=np.float32).reshape(D)

    nc = _get_nc(use_f32r=_use_f32r, with_bias=bool(np.any(bo)))
    in_maps = []
    for c in range(N_CORES):
        sl = slice(c * NB, (c + 1) * NB)
        in_maps.append({
            "query": np.ascontiguousarray(query[sl]),
            "context": np.ascontiguousarray(context[sl]),
            "Wq": Wq, "Wk": Wk, "Wv": Wv, "Wo": Wo, "bo": bo,
        })
    res = run_bass_kernel_spmd(nc, in_maps, core_ids=list(range(N_CORES)))
    return np.concatenate([res.results[c]["out"] for c in range(N_CORES)], axis=0)
